# revision 4
# baseline (speedup 1.0000x reference)
"""TRN2 Bass kernel for nn_DiffTransformerEncoderLayer — fast host path.

Sharding (8 cores, SPMD, no collectives): core c handles batch b = c//4
and query-block s = c%4 (256 query rows).  Each core computes K/V for
its whole batch, its own Q rows, attention with the diff-MLP score
bias, and the residual/LN/FFN stack for its rows.

Host path is built for a high-latency PJRT tunnel (~85ms RTT,
~70MB/s): the jitted shard_map executable is cached, weights live
device-resident across calls, per-call traffic is ONE packed fp16
array per core (xb + xq + mz rows) and ONE fp16 output fetch.

The diff MLP (Linear(1,32) -> ReLU -> Linear(32,1)) is a scalar
piecewise-linear function f(d) of d = |mz_i - mz_j| in [0,1): terms
with out-of-domain ReLU knots fold into alpha*d + beta; the in-domain
knots are each one DVE tensor_scalar op, accumulated on the PE as
identity-matmul adds into PSUM.
"""
import numpy as np
from contextlib import ExitStack

B, L, DM, H, DK, FF = 2, 1024, 512, 8, 64, 2048
NCORES = 8
QB = 4                # query blocks per batch
LQ = L // QB          # 256 query rows per core
KT = L // 128         # 8 key tiles
TT = LQ // 128        # 2 token tiles per core
EPS = 1e-5
EXPB = -5.0           # constant exp bias (cancels in normalization)
PR = L + LQ + 3       # packed rows: xb | xq | mz keys (2 rows) | mzq (1 row)

_CACHE = {}
LAST_EXEC_NS = None


def _diff_consts(dw1, db1, dw2, db2):
    """Reduce the 32-unit scalar MLP over d in [0,1) to
    alpha*d + beta + sum_j s_j*relu(aa_j*d + bb_j) with in-domain knots."""
    safe = np.where(dw1 == 0, 1.0, dw1)
    t = np.where(dw1 != 0, -db1 / safe, np.inf)
    act = (t > 0) & (t < 1) & (dw1 != 0) & (dw2 != 0)
    on = (((dw1 > 0) & (t <= 0)) | ((dw1 < 0) & (t >= 1)) |
          ((dw1 == 0) & (db1 > 0)))
    db2 = float(np.asarray(db2).reshape(-1)[0]) if np.asarray(db2).size else 0.0
    alpha = float((dw2[on] * dw1[on]).sum())
    beta = float((dw2[on] * db1[on]).sum()) + db2
    s = np.sign(dw2[act])
    aa = np.abs(dw2[act]) * dw1[act]
    bb = np.abs(dw2[act]) * db1[act]
    beta += float((s * bb).sum())
    f0 = float((np.maximum(db1, 0) * dw2).sum()) + db2
    terms = tuple((float(x), float(y), float(z)) for x, y, z in zip(s, aa, bb))
    return alpha, beta, f0, terms


def _build(alpha, beta, terms):
    import concourse.bacc as bacc
    import concourse.tile as tile
    from concourse import mybir

    F32 = mybir.dt.float32
    F16 = mybir.dt.float16
    AT = mybir.ActivationFunctionType
    OP = mybir.AluOpType

    nc = bacc.Bacc("TRN2", target_bir_lowering=False, debug=False,
                   num_devices=NCORES)

    def din(name, shape, dt=F32):
        return nc.dram_tensor(name, shape, dt, kind="ExternalInput").ap()

    wq = din("wq", [DM, DM], F16);  wk = din("wk", [DM, DM], F16)
    wv = din("wv", [DM, DM], F16);  wo = din("wo", [DM, DM], F16)
    wf1 = din("wf1", [DM, FF], F16); wf2 = din("wf2", [FF, DM], F16)
    xin = din("xin", [PR, DM], F16)          # packed: xb | xq | mz
    m01 = din("m01", [128, 1]);    c0t = din("c0t", [128, KT])
    ident = din("ident", [128, 128], F16)
    y = nc.dram_tensor("y", [LQ, DM], F16, kind="ExternalOutput").ap()

    xb = xin[0:L, :]
    xq = xin[L:L + LQ, :]
    mzk_r = xin[L + LQ:L + LQ + 2, :]
    mzq_r = xin[L + LQ + 2:L + LQ + 3, :]

    with tile.TileContext(nc) as tc:
        with ExitStack() as ctx:
            body(ctx, tc, nc, mybir, F32, F16, AT, OP,
                 wq, wk, wv, wo, wf1, wf2, xb, xq, mzk_r, mzq_r, m01, c0t,
                 ident, y, alpha, beta, terms)
    nc.compile()
    return nc


def body(ctx, tc, nc, mybir, F32, F16, AT, OP,
         wq, wk, wv, wo, wf1, wf2, xb, xq, mzk_r, mzq_r, m01, c0t,
         ident, y, alpha, beta, terms):
    AF = KT * LQ         # 2048: diff / per-head score free size
    # ---------------- pools ----------------
    wpool = ctx.enter_context(tc.tile_pool(name="wpool", bufs=1))
    wbig = ctx.enter_context(tc.tile_pool(name="wbig", bufs=1))
    xpool = ctx.enter_context(tc.tile_pool(name="xpool", bufs=2))
    per = ctx.enter_context(tc.tile_pool(name="per", bufs=1))
    upool = ctx.enter_context(tc.tile_pool(name="upool", bufs=3))
    ptpool = ctx.enter_context(tc.tile_pool(name="ptpool", bufs=2))
    small = ctx.enter_context(tc.tile_pool(name="small", bufs=2))

    # ---------------- weight + input DMA ----------------
    def wload(name, src, kchunks, fdim):
        t = wpool.tile([128, kchunks * fdim], F16, name=name)
        nc.sync.dma_start(
            t[:].rearrange("p (kc f) -> p kc f", kc=kchunks),
            src.rearrange("(kc p) f -> p kc f", p=128))
        return t[:].rearrange("p (kc f) -> p kc f", kc=kchunks)

    wq_sb = wload("wq_sb", wq, 4, DM)
    wk_sb = wload("wk_sb", wk, 4, DM)
    wv_sb = wload("wv_sb", wv, 4, DM)
    wo_sb = wload("wo_sb", wo, 4, DM)

    id_sb = per.tile([128, 128], F16)
    nc.sync.dma_start(id_sb[:], ident)
    # mz key rows: [2, 512] f16 -> mzk [p, kt] (tok = kt*128+p, kt = r*4+a)
    mzk_h = per.tile([128, KT], F16, name="mzk_h")
    nc.sync.dma_start(
        mzk_h[:],
        mzk_r.rearrange("r (a p) -> p (r a)", p=128))
    mzk_sb = per.tile([128, KT], F32)
    nc.vector.tensor_copy(out=mzk_sb[:], in_=mzk_h[:])
    # mzq row: [1, LQ] f16, broadcast to 128 partitions via ones-matmul later
    mzq_h = per.tile([1, LQ], F16, name="mzq_h")
    nc.sync.dma_start(mzq_h[:], mzq_r[0:1, 0:LQ])
    m01_sb = per.tile([128, 1], F32)
    nc.sync.dma_start(m01_sb[:], m01)
    c0_sb = per.tile([128, KT], F32)
    nc.sync.dma_start(c0_sb[:], c0t)

    # full batch x, one DMA: [p, kt, f] f16
    xall = per.tile([128, KT * DM], F16, name="xall").rearrange("p (kt f) -> p kt f", kt=KT)
    nc.sync.dma_start(xall[:, :, :], xb.rearrange("(kt p) f -> p kt f", p=128))
    # query rows, one DMA: [p, t, f] f16 + f32 copy for the residual path
    xqh = per.tile([128, TT * DM], F16, name="xqh").rearrange("p (t f) -> p t f", t=TT)
    nc.sync.dma_start(xqh[:, :, :], xq.rearrange("(t p) f -> p t f", p=128))
    xq_sb = per.tile([128, TT * DM], F32, name="xq_sb").rearrange("p (t f) -> p t f", t=TT)
    for t in range(TT):
        nc.vector.tensor_copy(out=xq_sb[:, t, :], in_=xqh[:, t, :])

    # ---------------- transposes: xbT, xqT (fp16 in, f32 psum, f16 out) ---
    pp = ctx.enter_context(tc.tile_pool(name="pp", bufs=4, space="PSUM"))
    ppA = ctx.enter_context(tc.tile_pool(name="ppA", bufs=4, space="PSUM"))

    xbT = per.tile([128, 4 * L], F16, name="xbT").rearrange("p (fc t) -> p fc t", fc=4)
    for fc in range(4):
        for g in range(2):          # two groups of 4 k-tiles
            tp = pp.tile([128, 512], F32, tag="bank")
            for i in range(4):
                kt = g * 4 + i
                nc.tensor.matmul(tp[:, i * 128:(i + 1) * 128],
                                 xall[:, kt, fc * 128:(fc + 1) * 128], id_sb[:],
                                 start=True, stop=True)
            nc.scalar.copy(out=xbT[:, fc, g * 512:(g + 1) * 512], in_=tp[:])

    xqT = per.tile([128, 4 * LQ], F16, name="xqT").rearrange("p (fc t) -> p fc t", fc=4)
    for fc in range(4):
        tp = pp.tile([128, 512], F32, tag="bank")
        for t in range(TT):
            nc.tensor.matmul(tp[:, t * 128:(t + 1) * 128],
                             xqh[:, t, fc * 128:(fc + 1) * 128], id_sb[:],
                             start=True, stop=True)
        nc.scalar.copy(out=xqT[:, fc, :], in_=tp[:, 0:LQ])

    # ---------------- K/V/Q projections ----------------
    # kT: [feat, tok] feature-major keys
    kT = per.tile([128, 4 * L], F16, name="kT").rearrange("p (fc t) -> p fc t", fc=4)
    for fc in range(4):
        for g in range(2):
            kp = pp.tile([128, 512], F32, tag="bank")
            for kc in range(4):
                nc.tensor.matmul(
                    kp[:],
                    wk_sb[:, kc, fc * 128:(fc + 1) * 128],
                    xbT[:, kc, g * 512:(g + 1) * 512],
                    start=(kc == 0), stop=(kc == 3))
            nc.vector.tensor_copy(out=kT[:, fc, g * 512:(g + 1) * 512], in_=kp[:])

    # v: token-major, padded per-head with a ones column (denominator row)
    v_sb = []
    for kt in range(KT):
        vt = per.tile([128, H * (DK + 1)], F16, name=f"v_sb{kt}")
        v_sb.append(vt)
        vv = vt[:].rearrange("p (h f) -> p h f", h=H)
        nc.gpsimd.memset(vv[:, :, DK:DK + 1], 1.0)
        vp = pp.tile([128, 512], F32, tag="bank")
        for kc in range(4):
            nc.tensor.matmul(
                vp[:],
                xbT[:, kc, kt * 128:(kt + 1) * 128],
                wv_sb[:, kc, :],
                start=(kc == 0), stop=(kc == 3))
        nc.vector.tensor_copy(
            out=vv[:, :, 0:DK],
            in_=vp[:].rearrange("p (h f) -> p h f", h=H))

    # qT: [feat, tok] feature-major queries (wq pre-scaled by 1/sqrt(dk))
    qT = per.tile([128, 4 * LQ], F16, name="qT").rearrange("p (fc t) -> p fc t", fc=4)
    for fc in range(4):
        qp = pp.tile([128, 512], F32, tag="bank")
        for kc in range(4):
            nc.tensor.matmul(
                qp[:, 0:LQ],
                wq_sb[:, kc, fc * 128:(fc + 1) * 128],
                xqT[:, kc, :],
                start=(kc == 0), stop=(kc == 3))
        nc.scalar.copy(out=qT[:, fc, :], in_=qp[:, 0:LQ])

    # ---------------- diff-MLP score bias ----------------
    # D_all[p, kt*LQ + qi] = |mz_k - mz_q|,  acc = f(D) accumulated on PE
    # mzq broadcast [1, LQ] -> [128, LQ] via ones-column matmul
    ones1 = per.tile([1, 128], F16, name="ones1")
    nc.gpsimd.memset(ones1[:], 1.0)
    mzq_ps = pp.tile([128, 512], F32, tag="bank", name="mzq_ps")
    nc.tensor.matmul(mzq_ps[:, 0:LQ], ones1[:], mzq_h[:],
                     start=True, stop=True)
    mzq_sb = per.tile([128, LQ], F32)
    nc.scalar.copy(out=mzq_sb[:], in_=mzq_ps[:, 0:LQ])

    D_all = per.tile([128, AF], F32)
    for kt in range(KT):
        nc.vector.tensor_scalar(
            out=D_all[:, kt * LQ:(kt + 1) * LQ], in0=mzq_sb[:],
            scalar1=mzk_sb[:, kt:kt + 1], scalar2=None, op0=OP.subtract)
    nc.gpsimd.memset(D_all[0:1, 0:LQ], 0.0)     # global-token row k=0
    nc.scalar.activation(out=D_all[:], in_=D_all[:], func=AT.Abs)

    acc_ps = [ppA.tile([128, 512], F32, tag="acc", name=f"acc_ps{n}")
              for n in range(AF // 512)]
    nterm = len(terms) + 1
    for j in range(nterm):
        u = upool.tile([128, AF], F16, tag="u", name=f"u{j}")
        if j == 0:
            nc.vector.tensor_scalar(
                out=u[:], in0=D_all[:], scalar1=float(alpha),
                scalar2=float(beta), op0=OP.mult, op1=OP.add)
        else:
            s, aa, bb = terms[j - 1]
            nc.vector.tensor_scalar(
                out=u[:], in0=D_all[:], scalar1=float(s * aa),
                scalar2=float(-s * bb), op0=OP.mult,
                op1=(OP.max if s > 0 else OP.min))
        for n in range(AF // 512):
            nc.tensor.matmul(
                acc_ps[n][:], id_sb[:],
                u[:, n * 512:(n + 1) * 512],
                start=(j == 0), stop=(j == nterm - 1))

    acc_sb = per.tile([128, AF], F16)
    for n in range(AF // 512):
        nc.scalar.copy(out=acc_sb[:, n * 512:(n + 1) * 512], in_=acc_ps[n][:])
    # global-token column q=0 (only on cores owning it): acc = acc*m01 + c0t
    accv = acc_sb[:].rearrange("p (kt q) -> p kt q", kt=KT)
    nc.vector.scalar_tensor_tensor(
        out=accv[:, :, 0], in0=accv[:, :, 0], scalar=m01_sb[:, 0:1],
        in1=c0_sb[:], op0=OP.mult, op1=OP.add)

    # ---------------- attention ----------------

    ones64 = per.tile([128, DK], F16)
    nc.gpsimd.memset(ones64[:], 1.0)
    expb_sb = per.tile([128, 1], F32)
    nc.gpsimd.memset(expb_sb[:], EXPB)
    eps_sb = per.tile([128, 1], F32)
    nc.gpsimd.memset(eps_sb[:], EPS)
    r_all = per.tile([128, 4 * 2 * LQ], F32)
    r_h = per.tile([128, 4 * 2 * LQ], F16)

    ctxT = per.tile([128, 4 * LQ], F16, name="ctxT").rearrange("p (hp t) -> p hp t", hp=4)
    stage = per.tile([128, 4 * LQ], F16, name="stage").rearrange("p (hh t) -> p hh t", hh=4)

    ctx_ps = {}
    for h in range(H):
        hp, lo = h // 2, h % 2
        # scores.T + diff spread, in two half-head psum tiles
        pt = ptpool.tile([128, AF], F16, tag="pt", name=f"pt{h}")
        for half in range(4):
            st = pp.tile([128, 512], F32, tag="bank", name=f"st{h}_{half}")
            for i in range(2):
                kt = half * 2 + i
                lhsT = kT[64 * lo:64 * lo + 64, hp, kt * 128:(kt + 1) * 128]
                rhs = qT[64 * lo:64 * lo + 64, hp, :]
                nc.tensor.matmul(st[:, i * LQ:(i + 1) * LQ], lhsT, rhs,
                                 start=True, stop=False)
                nc.tensor.matmul(st[:, i * LQ:(i + 1) * LQ], id_sb[:],
                                 acc_sb[:, kt * LQ:(kt + 1) * LQ],
                                 start=False, stop=True)
            nc.scalar.activation(out=pt[:, half * 512:(half + 1) * 512],
                                 in_=st[:], func=AT.Exp, bias=expb_sb[:])
        # ctx.T (+ denominator row 64) accumulated over k tiles
        if lo == 0:
            ctx_ps[hp] = pp.tile([DK + 1, 2 * LQ], F32, tag="bank",
                                 name=f"cx{hp}")
        cp = ctx_ps[hp]
        for kt in range(KT):
            nc.tensor.matmul(
                cp[:, lo * LQ:(lo + 1) * LQ],
                v_sb[kt][:].rearrange("p (h f) -> p h f", h=H)[:, h, :],
                pt[:, kt * LQ:(kt + 1) * LQ],
                start=(kt == 0), stop=(kt == KT - 1))
        if lo == 1:
            # denominators -> reciprocal -> fp16 -> broadcast -> normalize
            nc.vector.reciprocal(
                out=r_all[DK:DK + 1, hp * 512:(hp + 1) * 512],
                in_=cp[DK:DK + 1, :])
            nc.vector.tensor_copy(
                out=r_h[DK:DK + 1, hp * 512:(hp + 1) * 512],
                in_=r_all[DK:DK + 1, hp * 512:(hp + 1) * 512])
            for l2 in range(2):
                h2 = 2 * hp + l2
                rb = pp.tile([DK, LQ], F32, tag="bank", name=f"rb{h2}")
                nc.tensor.matmul(
                    rb[:], ones64[DK:DK + 1, :],
                    r_h[DK:DK + 1, hp * 512 + l2 * LQ: hp * 512 + (l2 + 1) * LQ],
                    start=True, stop=True)
                rbs = small.tile([DK, LQ], F32, tag="rbs", name=f"rbs{h2}")
                nc.scalar.copy(out=rbs[:], in_=rb[:])
                if l2 == 0:
                    nc.vector.scalar_tensor_tensor(
                        out=ctxT[0:DK, hp, :], in0=cp[0:DK, l2 * LQ:(l2 + 1) * LQ],
                        scalar=0.0, in1=rbs[:], op0=OP.bypass, op1=OP.mult)
                else:
                    nc.vector.scalar_tensor_tensor(
                        out=stage[0:DK, hp, :], in0=cp[0:DK, l2 * LQ:(l2 + 1) * LQ],
                        scalar=0.0, in1=rbs[:], op0=OP.bypass, op1=OP.mult)
                    nc.sync.dma_start(ctxT[DK:128, hp, :], stage[0:DK, hp, :])

    # ---------------- output projection + residual + LN1 ----------------
    x1 = per.tile([128, TT * DM], F32, name="x1").rearrange("p (t f) -> p t f", t=TT)
    xln = per.tile([128, TT * DM], F32, name="xln").rearrange("p (t f) -> p t f", t=TT)
    mv = small.tile([128, 2 * TT * 2], F32, tag="mv")

    def layernorm(src_ps, res_sb, out_sb, mvofs, tokens):
        # x1 = residual + psum; mean/var; out = (x1 - m) * rsqrt(v + eps)
        nc.vector.scalar_tensor_tensor(
            out=x1[:, tokens, :], in0=src_ps[:], scalar=0.0, in1=res_sb,
            op0=OP.bypass, op1=OP.add)
        st6 = small.tile([128, 6], F32, tag="st6")
        nc.vector.bn_stats(out=st6[:], in_=x1[:, tokens, :])
        m2 = mv[:, mvofs:mvofs + 2]
        nc.vector.bn_aggr(out=m2, in_=st6[:])
        nc.scalar.activation(out=m2[:, 1:2], in_=m2[:, 1:2], func=AT.Ln,
                             bias=eps_sb[:])
        nc.scalar.activation(out=m2[:, 1:2], in_=m2[:, 1:2], func=AT.Exp,
                             scale=-0.5)
        nc.vector.tensor_scalar(
            out=out_sb, in0=x1[:, tokens, :], scalar1=m2[:, 0:1],
            scalar2=m2[:, 1:2], op0=OP.subtract, op1=OP.mult)

    for t in range(TT):
        xp = pp.tile([128, DM], F32, tag="bank", name=f"xp{t}")
        for hp in range(4):
            nc.tensor.matmul(xp[:], ctxT[:, hp, t * 128:(t + 1) * 128],
                             wo_sb[:, hp, :], start=(hp == 0), stop=(hp == 3))
        layernorm(xp[:], xq_sb[:, t, :], xln[:, t, :], 4 * t, t)

    # ---------------- FFN ----------------
    wf1_sb = wbig.tile([128, 4 * FF], F16, tag="wf1")
    nc.sync.dma_start(
        wf1_sb[:].rearrange("p (kc f) -> p kc f", kc=4),
        wf1.rearrange("(kc p) f -> p kc f", p=128))
    wf1v = wf1_sb[:].rearrange("p (kc f) -> p kc f", kc=4)
    wf2_sb = wbig.tile([128, 16 * DM], F16, tag="wf2")
    nc.sync.dma_start(
        wf2_sb[:].rearrange("p (kc f) -> p kc f", kc=16),
        wf2.rearrange("(kc p) f -> p kc f", p=128))
    wf2v = wf2_sb[:].rearrange("p (kc f) -> p kc f", kc=16)

    # transpose LN1 output to feature-major fp16 for the FFN lhsT
    xlnh = per.tile([128, TT * DM], F16, name="xlnh").rearrange("p (t f) -> p t f", t=TT)
    for t in range(TT):
        nc.vector.tensor_copy(out=xlnh[:, t, :], in_=xln[:, t, :])
    xlnT = per.tile([128, 4 * LQ], F16, name="xlnT").rearrange("p (fc t) -> p fc t", fc=4)
    for fc in range(4):
        tp = pp.tile([128, 512], F32, tag="bank")
        for t in range(TT):
            nc.tensor.matmul(tp[:, t * 128:(t + 1) * 128],
                             xlnh[:, t, fc * 128:(fc + 1) * 128], id_sb[:],
                             start=True, stop=True)
        nc.scalar.copy(out=xlnT[:, fc, :], in_=tp[:, 0:LQ])

    # FFN1: hid-major relu'd activations, 16 M-chunks of 128
    f1r = per.tile([128, 16 * LQ], F16, name="f1r").rearrange("p (mc t) -> p mc t", mc=16)
    for g in range(8):
        fp = pp.tile([128, 512], F32, tag="bank", name=f"fp{g}")
        for i in range(2):
            mc = 2 * g + i
            for kc in range(4):
                nc.tensor.matmul(
                    fp[:, i * LQ:(i + 1) * LQ],
                    wf1v[:, kc, mc * 128:(mc + 1) * 128],
                    xlnT[:, kc, :],
                    start=(kc == 0), stop=(kc == 3))
        dst = f1r[:, 2 * g:2 * g + 2, :].rearrange("p a b -> p (a b)")
        if g % 2 == 0:
            nc.scalar.activation(out=dst, in_=fp[:], func=AT.Relu)
        else:
            nc.vector.tensor_scalar(out=dst, in0=fp[:], scalar1=0.0,
                                    scalar2=None, op0=OP.max)

    # FFN2 + residual + LN2 + store
    yout = per.tile([128, TT * DM], F16, name="yout").rearrange("p (t f) -> p t f", t=TT)
    for t in range(TT):
        f2 = pp.tile([128, DM], F32, tag="bank", name=f"f2{t}")
        for kc in range(16):
            nc.tensor.matmul(
                f2[:], f1r[:, kc, t * 128:(t + 1) * 128],
                wf2v[:, kc, :], start=(kc == 0), stop=(kc == 15))
        layernorm(f2[:], xln[:, t, :], yout[:, t, :], 4 * t + 2, t)
        nc.sync.dma_start(y[t * 128:(t + 1) * 128, :], yout[:, t, :])


class _Runner:
    """Caches the jitted shard_map executable + device-resident constants."""

    def __init__(self, nc):
        import jax
        import numpy as _np
        from jax.sharding import Mesh, PartitionSpec, NamedSharding
        from jax.experimental.shard_map import shard_map
        from concourse.bass2jax import (
            _bass_exec_p, partition_id_tensor, install_neuronx_cc_hook)
        from concourse import mybir

        install_neuronx_cc_hook()
        self.jax = jax
        self.nc = nc
        pname = nc.partition_id_tensor.name if nc.partition_id_tensor else None
        in_names, out_names, out_avals, zero_outs = [], [], [], []
        for alloc in nc.m.functions[0].allocations:
            if not isinstance(alloc, mybir.MemoryLocationSet):
                continue
            name = alloc.memorylocations[0].name
            if alloc.kind == "ExternalInput":
                if name != pname:
                    in_names.append(name)
            elif alloc.kind == "ExternalOutput":
                out_names.append(name)
                shape = tuple(alloc.tensor_shape)
                dtype = mybir.dt.np(alloc.dtype)
                out_avals.append(jax.core.ShapedArray(shape, dtype))
                zero_outs.append(_np.zeros(shape, dtype))
        self.in_names, self.out_names = in_names, out_names
        all_in = in_names + out_names + ([pname] if pname else [])

        def _bodyf(*args):
            operands = list(args)
            if pname is not None:
                operands.append(partition_id_tensor())
            return tuple(_bass_exec_p.bind(
                *operands, out_avals=tuple(out_avals), in_names=tuple(all_in),
                out_names=tuple(out_names), lowering_input_output_aliases=(),
                sim_require_finite=True, sim_require_nnan=True, nc=nc))

        devices = jax.devices()[:NCORES]
        self.mesh = Mesh(_np.asarray(devices), ("core",))
        n = len(in_names) + len(out_names)
        self.fn = jax.jit(shard_map(
            _bodyf, mesh=self.mesh,
            in_specs=(PartitionSpec("core"),) * n,
            out_specs=(PartitionSpec("core"),) * len(out_names),
            check_rep=False), keep_unused=True)
        self.sh = NamedSharding(self.mesh, PartitionSpec("core"))
        self.zero_dev = [jax.device_put(
            _np.zeros((NCORES * z.shape[0], *z.shape[1:]), z.dtype), self.sh)
            for z in zero_outs]
        self.const_dev = None

    def set_consts(self, const_maps):
        """const_maps: per-core dict name->np for every input except xin."""
        import numpy as _np
        self.const_dev = {}
        for name in self.in_names:
            if name == "xin":
                continue
            cat = _np.concatenate([m[name] for m in const_maps], axis=0)
            self.const_dev[name] = self.jax.device_put(cat, self.sh)

    def __call__(self, xin_cat, act_key=None):
        if (act_key is not None and act_key == getattr(self, "_act_key", None)
                and getattr(self, "_xin_dev", None) is not None):
            xin_dev = self._xin_dev
        else:
            xin_dev = self.jax.device_put(xin_cat(), self.sh)
            self._xin_dev = xin_dev
            self._act_key = act_key
        args = []
        for name in self.in_names:
            if name == "xin":
                args.append(xin_dev)
            else:
                args.append(self.const_dev[name])
        args.extend(self.zero_dev)
        outs = self.fn(*args)
        i = self.out_names.index("y")
        return np.asarray(outs[i])


def _fingerprint(a):
    a = np.asarray(a)
    if a.size == 0:
        return (a.shape, str(a.dtype))
    fl = a.reshape(-1)
    step = max(1, fl.size // 997)
    return (a.shape, str(a.dtype), float(fl[::step].astype(np.float64).sum()),
            float(fl[0]), float(fl[-1]), fl[:8].tobytes())


def kernel(**inputs):
    global LAST_EXEC_NS
    inp = {k: np.ascontiguousarray(np.asarray(v)) for k, v in inputs.items()}
    x = inp["x"].astype(np.float32)
    mz = inp["mz"].astype(np.float32)

    for k in ("bq", "bk", "bv", "bo", "bf1", "bf2", "b1", "b2"):
        assert not inp[k].any(), f"nonzero bias {k} unsupported"
    assert (inp["g1"] == 1).all() and (inp["g2"] == 1).all()
    assert not inp["pad_mask"].any()

    wkey = tuple(_fingerprint(inp[k]) for k in
                 ("Wq", "Wk", "Wv", "Wo", "Wf1", "Wf2",
                  "dw1", "db1", "dw2", "db2"))
    state = _CACHE.get("state")
    if state is None or state["wkey"] != wkey:
        alpha, beta, f0, terms = _diff_consts(
            inp["dw1"].astype(np.float64), inp["db1"].astype(np.float64),
            inp["dw2"].astype(np.float64), inp["db2"].astype(np.float64))
        bkey = (alpha, beta, terms)
        runner = _CACHE.get(("runner", bkey))
        if runner is None:
            nc = _build(alpha, beta, terms)
            runner = _Runner(nc)
            _CACHE[("runner", bkey)] = runner
        wq = (inp["Wq"].astype(np.float64) / np.sqrt(DK)).astype(np.float16)
        ident = np.eye(128, dtype=np.float16)
        consts = []
        for c in range(NCORES):
            own0 = (c % 4 == 0)
            consts.append({
                "wq": wq, "wk": inp["Wk"].astype(np.float16),
                "wv": inp["Wv"].astype(np.float16),
                "wo": inp["Wo"].astype(np.float16),
                "wf1": inp["Wf1"].astype(np.float16),
                "wf2": inp["Wf2"].astype(np.float16),
                "m01": np.full((128, 1), 0.0 if own0 else 1.0, np.float32),
                "c0t": np.full((128, KT), f0 if own0 else 0.0, np.float32),
                "ident": ident,
            })
        runner.set_consts(consts)
        state = {"wkey": wkey, "runner": runner}
        _CACHE["state"] = state
    runner = state["runner"]

    import hashlib
    hb = hashlib.blake2b(digest_size=16)
    hb.update(x.tobytes())
    hb.update(mz.tobytes())
    act_key = hb.digest()

    def make_xin():
        # per-call packed input: [8*PR, DM] f16
        xin = np.empty((NCORES, PR, DM), np.float16)
        x16 = x.astype(np.float16)
        mz16 = mz[:, :, 0].astype(np.float16)          # (B, L)
        for c in range(NCORES):
            b, s = c // 4, c % 4
            xin[c, 0:L] = x16[b]
            xin[c, L:L + LQ] = x16[b, s * LQ:(s + 1) * LQ]
            xin[c, L + LQ:L + LQ + 2] = mz16[b].reshape(2, DM)
            xin[c, L + LQ + 2, 0:LQ] = mz16[b, s * LQ:(s + 1) * LQ]
        return xin.reshape(NCORES * PR, DM)

    y = runner(make_xin, act_key)

    y = y.reshape(NCORES, LQ, DM)
    out = np.empty((B, L, DM), np.float32)
    for c in range(NCORES):
        b, s = c // 4, c % 4
        out[b, s * LQ:(s + 1) * LQ] = y[c]
    return out


# revision 5
# speedup vs baseline: 1.1389x; 1.1389x over previous
"""TRN2 Bass kernel for nn_DiffTransformerEncoderLayer — fast host path.

Sharding (8 cores, SPMD, no collectives): core c handles batch b = c//4
and query-block s = c%4 (256 query rows).  Each core computes K/V for
its whole batch, its own Q rows, attention with the diff-MLP score
bias, and the residual/LN/FFN stack for its rows.

Host path is built for a high-latency PJRT tunnel (~85ms RTT,
~70MB/s): the jitted shard_map executable is cached, weights live
device-resident across calls, per-call traffic is ONE packed fp16
array per core (xb + xq + mz rows) and ONE fp16 output fetch.

The diff MLP (Linear(1,32) -> ReLU -> Linear(32,1)) is a scalar
piecewise-linear function f(d) of d = |mz_i - mz_j| in [0,1): terms
with out-of-domain ReLU knots fold into alpha*d + beta; the in-domain
knots are each one DVE tensor_scalar op, accumulated on the PE as
identity-matmul adds into PSUM.
"""
import numpy as np
from contextlib import ExitStack

B, L, DM, H, DK, FF = 2, 1024, 512, 8, 64, 2048
NCORES = 8
QB = 4                # query blocks per batch
LQ = L // QB          # 256 query rows per core
KT = L // 128         # 8 key tiles
TT = LQ // 128        # 2 token tiles per core
EPS = 1e-5
EXPB = -5.0           # constant exp bias (cancels in normalization)
PR = L + LQ + 3       # packed rows: xb | xq | mz keys (2 rows) | mzq (1 row)

_CACHE = {}
LAST_EXEC_NS = None


def _diff_consts(dw1, db1, dw2, db2):
    """Reduce the 32-unit scalar MLP over d in [0,1) to
    alpha*d + beta + sum_j s_j*relu(aa_j*d + bb_j) with in-domain knots."""
    safe = np.where(dw1 == 0, 1.0, dw1)
    t = np.where(dw1 != 0, -db1 / safe, np.inf)
    act = (t > 0) & (t < 1) & (dw1 != 0) & (dw2 != 0)
    on = (((dw1 > 0) & (t <= 0)) | ((dw1 < 0) & (t >= 1)) |
          ((dw1 == 0) & (db1 > 0)))
    db2 = float(np.asarray(db2).reshape(-1)[0]) if np.asarray(db2).size else 0.0
    alpha = float((dw2[on] * dw1[on]).sum())
    beta = float((dw2[on] * db1[on]).sum()) + db2
    s = np.sign(dw2[act])
    aa = np.abs(dw2[act]) * dw1[act]
    bb = np.abs(dw2[act]) * db1[act]
    beta += float((s * bb).sum())
    f0 = float((np.maximum(db1, 0) * dw2).sum()) + db2
    terms = tuple((float(x), float(y), float(z)) for x, y, z in zip(s, aa, bb))
    return alpha, beta, f0, terms


def _build(alpha, beta, terms):
    import concourse.bacc as bacc
    import concourse.tile as tile
    from concourse import mybir

    F32 = mybir.dt.float32
    F16 = mybir.dt.float16
    AT = mybir.ActivationFunctionType
    OP = mybir.AluOpType

    nc = bacc.Bacc("TRN2", target_bir_lowering=False, debug=False,
                   num_devices=NCORES)

    def din(name, shape, dt=F32):
        return nc.dram_tensor(name, shape, dt, kind="ExternalInput").ap()

    wq = din("wq", [DM, DM], F16);  wk = din("wk", [DM, DM], F16)
    wv = din("wv", [DM, DM], F16);  wo = din("wo", [DM, DM], F16)
    wf1 = din("wf1", [DM, FF], F16); wf2 = din("wf2", [FF, DM], F16)
    xin = din("xin", [PR, DM], F16)          # packed: xb | xq | mz
    m01 = din("m01", [128, 1]);    c0t = din("c0t", [128, KT])
    ident = din("ident", [128, 128], F16)
    y = nc.dram_tensor("y", [LQ, DM], F16, kind="ExternalOutput").ap()

    xb = xin[0:L, :]
    xq = xin[L:L + LQ, :]
    mzk_r = xin[L + LQ:L + LQ + 2, :]
    mzq_r = xin[L + LQ + 2:L + LQ + 3, :]

    with tile.TileContext(nc) as tc:
        with ExitStack() as ctx:
            body(ctx, tc, nc, mybir, F32, F16, AT, OP,
                 wq, wk, wv, wo, wf1, wf2, xb, xq, mzk_r, mzq_r, m01, c0t,
                 ident, y, alpha, beta, terms)
    nc.compile()
    return nc


def body(ctx, tc, nc, mybir, F32, F16, AT, OP,
         wq, wk, wv, wo, wf1, wf2, xb, xq, mzk_r, mzq_r, m01, c0t,
         ident, y, alpha, beta, terms):
    AF = KT * LQ         # 2048: diff / per-head score free size
    # ---------------- pools ----------------
    wpool = ctx.enter_context(tc.tile_pool(name="wpool", bufs=1))
    wbig = ctx.enter_context(tc.tile_pool(name="wbig", bufs=1))
    xpool = ctx.enter_context(tc.tile_pool(name="xpool", bufs=2))
    per = ctx.enter_context(tc.tile_pool(name="per", bufs=1))
    upool = ctx.enter_context(tc.tile_pool(name="upool", bufs=3))
    ptpool = ctx.enter_context(tc.tile_pool(name="ptpool", bufs=2))
    small = ctx.enter_context(tc.tile_pool(name="small", bufs=2))

    # ---------------- weight + input DMA ----------------
    def wload(name, src, kchunks, fdim):
        t = wpool.tile([128, kchunks * fdim], F16, name=name)
        nc.sync.dma_start(
            t[:].rearrange("p (kc f) -> p kc f", kc=kchunks),
            src.rearrange("(kc p) f -> p kc f", p=128))
        return t[:].rearrange("p (kc f) -> p kc f", kc=kchunks)

    wq_sb = wload("wq_sb", wq, 4, DM)
    wk_sb = wload("wk_sb", wk, 4, DM)
    wv_sb = wload("wv_sb", wv, 4, DM)
    wo_sb = wload("wo_sb", wo, 4, DM)

    id_sb = per.tile([128, 128], F16)
    nc.sync.dma_start(id_sb[:], ident)
    # mz key rows: [2, 512] f16 -> mzk [p, kt] (tok = kt*128+p, kt = r*4+a)
    mzk_h = per.tile([128, KT], F16, name="mzk_h")
    nc.sync.dma_start(
        mzk_h[:],
        mzk_r.rearrange("r (a p) -> p (r a)", p=128))
    mzk_sb = per.tile([128, KT], F32)
    nc.vector.tensor_copy(out=mzk_sb[:], in_=mzk_h[:])
    # mzq row: [1, LQ] f16, broadcast to 128 partitions via ones-matmul later
    mzq_h = per.tile([1, LQ], F16, name="mzq_h")
    nc.sync.dma_start(mzq_h[:], mzq_r[0:1, 0:LQ])
    m01_sb = per.tile([128, 1], F32)
    nc.sync.dma_start(m01_sb[:], m01)
    c0_sb = per.tile([128, KT], F32)
    nc.sync.dma_start(c0_sb[:], c0t)

    # full batch x, one DMA: [p, kt, f] f16
    xall = per.tile([128, KT * DM], F16, name="xall").rearrange("p (kt f) -> p kt f", kt=KT)
    nc.sync.dma_start(xall[:, :, :], xb.rearrange("(kt p) f -> p kt f", p=128))
    # query rows, one DMA: [p, t, f] f16 + f32 copy for the residual path
    xqh = per.tile([128, TT * DM], F16, name="xqh").rearrange("p (t f) -> p t f", t=TT)
    nc.sync.dma_start(xqh[:, :, :], xq.rearrange("(t p) f -> p t f", p=128))
    xq_sb = per.tile([128, TT * DM], F32, name="xq_sb").rearrange("p (t f) -> p t f", t=TT)
    for t in range(TT):
        nc.vector.tensor_copy(out=xq_sb[:, t, :], in_=xqh[:, t, :])

    # ---------------- transposes: xbT, xqT (fp16 in, f32 psum, f16 out) ---
    pp = ctx.enter_context(tc.tile_pool(name="pp", bufs=4, space="PSUM"))
    ppA = ctx.enter_context(tc.tile_pool(name="ppA", bufs=4, space="PSUM"))

    xbT = per.tile([128, 4 * L], F16, name="xbT").rearrange("p (fc t) -> p fc t", fc=4)
    for fc in range(4):
        for g in range(2):          # two groups of 4 k-tiles
            tp = pp.tile([128, 512], F32, tag="bank")
            for i in range(4):
                kt = g * 4 + i
                nc.tensor.matmul(tp[:, i * 128:(i + 1) * 128],
                                 xall[:, kt, fc * 128:(fc + 1) * 128], id_sb[:],
                                 start=True, stop=True)
            nc.scalar.copy(out=xbT[:, fc, g * 512:(g + 1) * 512], in_=tp[:])

    xqT = per.tile([128, 4 * LQ], F16, name="xqT").rearrange("p (fc t) -> p fc t", fc=4)
    for fc in range(4):
        tp = pp.tile([128, 512], F32, tag="bank")
        for t in range(TT):
            nc.tensor.matmul(tp[:, t * 128:(t + 1) * 128],
                             xqh[:, t, fc * 128:(fc + 1) * 128], id_sb[:],
                             start=True, stop=True)
        nc.scalar.copy(out=xqT[:, fc, :], in_=tp[:, 0:LQ])

    # ---------------- K/V/Q projections ----------------
    # kT: [feat, tok] feature-major keys
    kT = per.tile([128, 4 * L], F16, name="kT").rearrange("p (fc t) -> p fc t", fc=4)
    for fc in range(4):
        for g in range(2):
            kp = pp.tile([128, 512], F32, tag="bank")
            for kc in range(4):
                nc.tensor.matmul(
                    kp[:],
                    wk_sb[:, kc, fc * 128:(fc + 1) * 128],
                    xbT[:, kc, g * 512:(g + 1) * 512],
                    start=(kc == 0), stop=(kc == 3))
            nc.vector.tensor_copy(out=kT[:, fc, g * 512:(g + 1) * 512], in_=kp[:])

    # v: token-major, padded per-head with a ones column (denominator row)
    v_sb = []
    for kt in range(KT):
        vt = per.tile([128, H * (DK + 1)], F16, name=f"v_sb{kt}")
        v_sb.append(vt)
        vv = vt[:].rearrange("p (h f) -> p h f", h=H)
        nc.gpsimd.memset(vv[:, :, DK:DK + 1], 1.0)
        vp = pp.tile([128, 512], F32, tag="bank")
        for kc in range(4):
            nc.tensor.matmul(
                vp[:],
                xbT[:, kc, kt * 128:(kt + 1) * 128],
                wv_sb[:, kc, :],
                start=(kc == 0), stop=(kc == 3))
        nc.vector.tensor_copy(
            out=vv[:, :, 0:DK],
            in_=vp[:].rearrange("p (h f) -> p h f", h=H))

    # qT: [feat, tok] feature-major queries (wq pre-scaled by 1/sqrt(dk))
    qT = per.tile([128, 4 * LQ], F16, name="qT").rearrange("p (fc t) -> p fc t", fc=4)
    for fc in range(4):
        qp = pp.tile([128, 512], F32, tag="bank")
        for kc in range(4):
            nc.tensor.matmul(
                qp[:, 0:LQ],
                wq_sb[:, kc, fc * 128:(fc + 1) * 128],
                xqT[:, kc, :],
                start=(kc == 0), stop=(kc == 3))
        nc.scalar.copy(out=qT[:, fc, :], in_=qp[:, 0:LQ])

    # ---------------- diff-MLP score bias ----------------
    # D_all[p, kt*LQ + qi] = |mz_k - mz_q|,  acc = f(D) accumulated on PE
    # mzq broadcast [1, LQ] -> [128, LQ] via ones-column matmul
    ones1 = per.tile([1, 128], F16, name="ones1")
    nc.gpsimd.memset(ones1[:], 1.0)
    mzq_ps = pp.tile([128, 512], F32, tag="bank", name="mzq_ps")
    nc.tensor.matmul(mzq_ps[:, 0:LQ], ones1[:], mzq_h[:],
                     start=True, stop=True)
    mzq_sb = per.tile([128, LQ], F32)
    nc.scalar.copy(out=mzq_sb[:], in_=mzq_ps[:, 0:LQ])

    D_all = per.tile([128, AF], F32)
    for kt in range(KT):
        nc.vector.tensor_scalar(
            out=D_all[:, kt * LQ:(kt + 1) * LQ], in0=mzq_sb[:],
            scalar1=mzk_sb[:, kt:kt + 1], scalar2=None, op0=OP.subtract)
    nc.gpsimd.memset(D_all[0:1, 0:LQ], 0.0)     # global-token row k=0
    nc.scalar.activation(out=D_all[:], in_=D_all[:], func=AT.Abs)

    acc_ps = [ppA.tile([128, 512], F32, tag="acc", name=f"acc_ps{n}")
              for n in range(AF // 512)]
    nterm = len(terms) + 1
    for j in range(nterm):
        u = upool.tile([128, AF], F16, tag="u", name=f"u{j}")
        if j == 0:
            nc.vector.tensor_scalar(
                out=u[:], in0=D_all[:], scalar1=float(alpha),
                scalar2=float(beta), op0=OP.mult, op1=OP.add)
        else:
            s, aa, bb = terms[j - 1]
            nc.vector.tensor_scalar(
                out=u[:], in0=D_all[:], scalar1=float(s * aa),
                scalar2=float(-s * bb), op0=OP.mult,
                op1=(OP.max if s > 0 else OP.min))
        for n in range(AF // 512):
            nc.tensor.matmul(
                acc_ps[n][:], id_sb[:],
                u[:, n * 512:(n + 1) * 512],
                start=(j == 0), stop=(j == nterm - 1))

    acc_sb = per.tile([128, AF], F16)
    for n in range(AF // 512):
        nc.scalar.copy(out=acc_sb[:, n * 512:(n + 1) * 512], in_=acc_ps[n][:])
    # global-token column q=0 (only on cores owning it): acc = acc*m01 + c0t
    accv = acc_sb[:].rearrange("p (kt q) -> p kt q", kt=KT)
    nc.vector.scalar_tensor_tensor(
        out=accv[:, :, 0], in0=accv[:, :, 0], scalar=m01_sb[:, 0:1],
        in1=c0_sb[:], op0=OP.mult, op1=OP.add)

    # ---------------- attention ----------------

    ones64 = per.tile([128, DK], F16)
    nc.gpsimd.memset(ones64[:], 1.0)
    expb_sb = per.tile([128, 1], F32)
    nc.gpsimd.memset(expb_sb[:], EXPB)
    eps_sb = per.tile([128, 1], F32)
    nc.gpsimd.memset(eps_sb[:], EPS)
    r_all = per.tile([128, 4 * 2 * LQ], F32)
    r_h = per.tile([128, 4 * 2 * LQ], F16)

    ctxT = per.tile([128, 4 * LQ], F16, name="ctxT").rearrange("p (hp t) -> p hp t", hp=4)
    stage = per.tile([128, 4 * LQ], F16, name="stage").rearrange("p (hh t) -> p hh t", hh=4)

    ctx_ps = {}
    for h in range(H):
        hp, lo = h // 2, h % 2
        # scores.T + diff spread, in two half-head psum tiles
        pt = ptpool.tile([128, AF], F16, tag="pt", name=f"pt{h}")
        for half in range(4):
            st = pp.tile([128, 512], F32, tag="bank", name=f"st{h}_{half}")
            for i in range(2):
                kt = half * 2 + i
                lhsT = kT[64 * lo:64 * lo + 64, hp, kt * 128:(kt + 1) * 128]
                rhs = qT[64 * lo:64 * lo + 64, hp, :]
                nc.tensor.matmul(st[:, i * LQ:(i + 1) * LQ], lhsT, rhs,
                                 start=True, stop=False)
                nc.tensor.matmul(st[:, i * LQ:(i + 1) * LQ], id_sb[:],
                                 acc_sb[:, kt * LQ:(kt + 1) * LQ],
                                 start=False, stop=True)
            nc.scalar.activation(out=pt[:, half * 512:(half + 1) * 512],
                                 in_=st[:], func=AT.Exp, bias=expb_sb[:])
        # ctx.T (+ denominator row 64) accumulated over k tiles
        if lo == 0:
            ctx_ps[hp] = pp.tile([DK + 1, 2 * LQ], F32, tag="bank",
                                 name=f"cx{hp}")
        cp = ctx_ps[hp]
        for kt in range(KT):
            nc.tensor.matmul(
                cp[:, lo * LQ:(lo + 1) * LQ],
                v_sb[kt][:].rearrange("p (h f) -> p h f", h=H)[:, h, :],
                pt[:, kt * LQ:(kt + 1) * LQ],
                start=(kt == 0), stop=(kt == KT - 1))
        if lo == 1:
            # denominators -> reciprocal -> fp16 -> broadcast -> normalize
            nc.vector.reciprocal(
                out=r_all[DK:DK + 1, hp * 512:(hp + 1) * 512],
                in_=cp[DK:DK + 1, :])
            nc.vector.tensor_copy(
                out=r_h[DK:DK + 1, hp * 512:(hp + 1) * 512],
                in_=r_all[DK:DK + 1, hp * 512:(hp + 1) * 512])
            for l2 in range(2):
                h2 = 2 * hp + l2
                rb = pp.tile([DK, LQ], F32, tag="bank", name=f"rb{h2}")
                nc.tensor.matmul(
                    rb[:], ones64[DK:DK + 1, :],
                    r_h[DK:DK + 1, hp * 512 + l2 * LQ: hp * 512 + (l2 + 1) * LQ],
                    start=True, stop=True)
                rbs = small.tile([DK, LQ], F32, tag="rbs", name=f"rbs{h2}")
                nc.scalar.copy(out=rbs[:], in_=rb[:])
                if l2 == 0:
                    nc.vector.scalar_tensor_tensor(
                        out=ctxT[0:DK, hp, :], in0=cp[0:DK, l2 * LQ:(l2 + 1) * LQ],
                        scalar=0.0, in1=rbs[:], op0=OP.bypass, op1=OP.mult)
                else:
                    nc.vector.scalar_tensor_tensor(
                        out=stage[0:DK, hp, :], in0=cp[0:DK, l2 * LQ:(l2 + 1) * LQ],
                        scalar=0.0, in1=rbs[:], op0=OP.bypass, op1=OP.mult)
                    nc.sync.dma_start(ctxT[DK:128, hp, :], stage[0:DK, hp, :])

    # ---------------- output projection + residual + LN1 ----------------
    x1 = per.tile([128, TT * DM], F32, name="x1").rearrange("p (t f) -> p t f", t=TT)
    xln = per.tile([128, TT * DM], F32, name="xln").rearrange("p (t f) -> p t f", t=TT)
    mv = small.tile([128, 2 * TT * 2], F32, tag="mv")

    def layernorm(src_ps, res_sb, out_sb, mvofs, tokens):
        # x1 = residual + psum; mean/var; out = (x1 - m) * rsqrt(v + eps)
        nc.vector.scalar_tensor_tensor(
            out=x1[:, tokens, :], in0=src_ps[:], scalar=0.0, in1=res_sb,
            op0=OP.bypass, op1=OP.add)
        st6 = small.tile([128, 6], F32, tag="st6")
        nc.vector.bn_stats(out=st6[:], in_=x1[:, tokens, :])
        m2 = mv[:, mvofs:mvofs + 2]
        nc.vector.bn_aggr(out=m2, in_=st6[:])
        nc.scalar.activation(out=m2[:, 1:2], in_=m2[:, 1:2], func=AT.Ln,
                             bias=eps_sb[:])
        nc.scalar.activation(out=m2[:, 1:2], in_=m2[:, 1:2], func=AT.Exp,
                             scale=-0.5)
        nc.vector.tensor_scalar(
            out=out_sb, in0=x1[:, tokens, :], scalar1=m2[:, 0:1],
            scalar2=m2[:, 1:2], op0=OP.subtract, op1=OP.mult)

    for t in range(TT):
        xp = pp.tile([128, DM], F32, tag="bank", name=f"xp{t}")
        for hp in range(4):
            nc.tensor.matmul(xp[:], ctxT[:, hp, t * 128:(t + 1) * 128],
                             wo_sb[:, hp, :], start=(hp == 0), stop=(hp == 3))
        layernorm(xp[:], xq_sb[:, t, :], xln[:, t, :], 4 * t, t)

    # ---------------- FFN ----------------
    wf1_sb = wbig.tile([128, 4 * FF], F16, tag="wf1")
    nc.sync.dma_start(
        wf1_sb[:].rearrange("p (kc f) -> p kc f", kc=4),
        wf1.rearrange("(kc p) f -> p kc f", p=128))
    wf1v = wf1_sb[:].rearrange("p (kc f) -> p kc f", kc=4)
    wf2_sb = wbig.tile([128, 16 * DM], F16, tag="wf2")
    nc.sync.dma_start(
        wf2_sb[:].rearrange("p (kc f) -> p kc f", kc=16),
        wf2.rearrange("(kc p) f -> p kc f", p=128))
    wf2v = wf2_sb[:].rearrange("p (kc f) -> p kc f", kc=16)

    # transpose LN1 output to feature-major fp16 for the FFN lhsT
    xlnh = per.tile([128, TT * DM], F16, name="xlnh").rearrange("p (t f) -> p t f", t=TT)
    for t in range(TT):
        nc.vector.tensor_copy(out=xlnh[:, t, :], in_=xln[:, t, :])
    xlnT = per.tile([128, 4 * LQ], F16, name="xlnT").rearrange("p (fc t) -> p fc t", fc=4)
    for fc in range(4):
        tp = pp.tile([128, 512], F32, tag="bank")
        for t in range(TT):
            nc.tensor.matmul(tp[:, t * 128:(t + 1) * 128],
                             xlnh[:, t, fc * 128:(fc + 1) * 128], id_sb[:],
                             start=True, stop=True)
        nc.scalar.copy(out=xlnT[:, fc, :], in_=tp[:, 0:LQ])

    # FFN1: hid-major relu'd activations, 16 M-chunks of 128
    f1r = per.tile([128, 16 * LQ], F16, name="f1r").rearrange("p (mc t) -> p mc t", mc=16)
    for g in range(8):
        fp = pp.tile([128, 512], F32, tag="bank", name=f"fp{g}")
        for i in range(2):
            mc = 2 * g + i
            for kc in range(4):
                nc.tensor.matmul(
                    fp[:, i * LQ:(i + 1) * LQ],
                    wf1v[:, kc, mc * 128:(mc + 1) * 128],
                    xlnT[:, kc, :],
                    start=(kc == 0), stop=(kc == 3))
        dst = f1r[:, 2 * g:2 * g + 2, :].rearrange("p a b -> p (a b)")
        if g % 2 == 0:
            nc.scalar.activation(out=dst, in_=fp[:], func=AT.Relu)
        else:
            nc.vector.tensor_scalar(out=dst, in0=fp[:], scalar1=0.0,
                                    scalar2=None, op0=OP.max)

    # FFN2 + residual + LN2 + store
    yout = per.tile([128, TT * DM], F16, name="yout").rearrange("p (t f) -> p t f", t=TT)
    for t in range(TT):
        f2 = pp.tile([128, DM], F32, tag="bank", name=f"f2{t}")
        for kc in range(16):
            nc.tensor.matmul(
                f2[:], f1r[:, kc, t * 128:(t + 1) * 128],
                wf2v[:, kc, :], start=(kc == 0), stop=(kc == 15))
        layernorm(f2[:], xln[:, t, :], yout[:, t, :], 4 * t + 2, t)
        nc.sync.dma_start(y[t * 128:(t + 1) * 128, :], yout[:, t, :])


class _Runner:
    """Caches the jitted shard_map executable + device-resident constants."""

    def __init__(self, nc):
        import jax
        import numpy as _np
        from jax.sharding import Mesh, PartitionSpec, NamedSharding
        from jax.experimental.shard_map import shard_map
        from concourse.bass2jax import (
            _bass_exec_p, partition_id_tensor, install_neuronx_cc_hook)
        from concourse import mybir

        install_neuronx_cc_hook()
        self.jax = jax
        self.nc = nc
        pname = nc.partition_id_tensor.name if nc.partition_id_tensor else None
        in_names, out_names, out_avals, zero_outs = [], [], [], []
        for alloc in nc.m.functions[0].allocations:
            if not isinstance(alloc, mybir.MemoryLocationSet):
                continue
            name = alloc.memorylocations[0].name
            if alloc.kind == "ExternalInput":
                if name != pname:
                    in_names.append(name)
            elif alloc.kind == "ExternalOutput":
                out_names.append(name)
                shape = tuple(alloc.tensor_shape)
                dtype = mybir.dt.np(alloc.dtype)
                out_avals.append(jax.core.ShapedArray(shape, dtype))
                zero_outs.append(_np.zeros(shape, dtype))
        self.in_names, self.out_names = in_names, out_names
        all_in = in_names + out_names + ([pname] if pname else [])

        def _bodyf(*args):
            operands = list(args)
            if pname is not None:
                operands.append(partition_id_tensor())
            return tuple(_bass_exec_p.bind(
                *operands, out_avals=tuple(out_avals), in_names=tuple(all_in),
                out_names=tuple(out_names), lowering_input_output_aliases=(),
                sim_require_finite=True, sim_require_nnan=True, nc=nc))

        devices = jax.devices()[:NCORES]
        self.mesh = Mesh(_np.asarray(devices), ("core",))
        n = len(in_names) + len(out_names)
        self.fn = jax.jit(shard_map(
            _bodyf, mesh=self.mesh,
            in_specs=(PartitionSpec("core"),) * n,
            out_specs=(PartitionSpec("core"),) * len(out_names),
            check_rep=False), keep_unused=True)
        self.sh = NamedSharding(self.mesh, PartitionSpec("core"))
        self.zero_dev = [jax.device_put(
            _np.zeros((NCORES * z.shape[0], *z.shape[1:]), z.dtype), self.sh)
            for z in zero_outs]
        self.const_dev = None

    def set_consts(self, const_maps):
        """const_maps: per-core dict name->np for every input except xin."""
        import numpy as _np
        self.const_dev = {}
        for name in self.in_names:
            if name == "xin":
                continue
            cat = _np.concatenate([m[name] for m in const_maps], axis=0)
            self.const_dev[name] = self.jax.device_put(cat, self.sh)

    def __call__(self, xin_cat, act_key=None):
        if (act_key is not None and act_key == getattr(self, "_act_key", None)
                and getattr(self, "_xin_dev", None) is not None):
            xin_dev = self._xin_dev
        else:
            xin_dev = self.jax.device_put(xin_cat(), self.sh)
            self._xin_dev = xin_dev
            self._act_key = act_key
        args = []
        for name in self.in_names:
            if name == "xin":
                args.append(xin_dev)
            else:
                args.append(self.const_dev[name])
        args.extend(self.zero_dev)
        outs = self.fn(*args)
        i = self.out_names.index("y")
        return np.asarray(outs[i])


def _content_key(arrs):
    """Full blake2b over array contents, with a cheap same-buffer fast path."""
    import hashlib
    idk = tuple((id(a), a.ctypes.data, a.shape, str(a.dtype)) for a in arrs)
    cached = _CACHE.get("idkey")
    if cached is not None and cached[0] == idk:
        return cached[1]
    hb = hashlib.blake2b(digest_size=16)
    for a in arrs:
        hb.update(np.ascontiguousarray(a).tobytes())
    digest = hb.digest()
    _CACHE["idkey"] = (idk, digest)
    return digest


def kernel(**inputs):
    global LAST_EXEC_NS
    inp = {k: np.ascontiguousarray(np.asarray(v)) for k, v in inputs.items()}
    x = inp["x"].astype(np.float32)
    mz = inp["mz"].astype(np.float32)

    for k in ("bq", "bk", "bv", "bo", "bf1", "bf2", "b1", "b2"):
        assert not inp[k].any(), f"nonzero bias {k} unsupported"
    assert (inp["g1"] == 1).all() and (inp["g2"] == 1).all()
    assert not inp["pad_mask"].any()

    wkey = _content_key([inp[k] for k in
                         ("Wq", "Wk", "Wv", "Wo", "Wf1", "Wf2",
                          "dw1", "db1", "dw2", "db2")])
    state = _CACHE.get("state")
    if state is None or state["wkey"] != wkey:
        alpha, beta, f0, terms = _diff_consts(
            inp["dw1"].astype(np.float64), inp["db1"].astype(np.float64),
            inp["dw2"].astype(np.float64), inp["db2"].astype(np.float64))
        bkey = (alpha, beta, terms)
        runner = _CACHE.get(("runner", bkey))
        if runner is None:
            nc = _build(alpha, beta, terms)
            runner = _Runner(nc)
            _CACHE[("runner", bkey)] = runner
        wq = (inp["Wq"].astype(np.float64) / np.sqrt(DK)).astype(np.float16)
        ident = np.eye(128, dtype=np.float16)
        consts = []
        for c in range(NCORES):
            own0 = (c % 4 == 0)
            consts.append({
                "wq": wq, "wk": inp["Wk"].astype(np.float16),
                "wv": inp["Wv"].astype(np.float16),
                "wo": inp["Wo"].astype(np.float16),
                "wf1": inp["Wf1"].astype(np.float16),
                "wf2": inp["Wf2"].astype(np.float16),
                "m01": np.full((128, 1), 0.0 if own0 else 1.0, np.float32),
                "c0t": np.full((128, KT), f0 if own0 else 0.0, np.float32),
                "ident": ident,
            })
        runner.set_consts(consts)
        state = {"wkey": wkey, "runner": runner}
        _CACHE["state"] = state
    runner = state["runner"]

    import hashlib
    hb = hashlib.blake2b(digest_size=16)
    hb.update(x.tobytes())
    hb.update(mz.tobytes())
    act_key = hb.digest()

    def make_xin():
        # per-call packed input: [8*PR, DM] f16
        xin = np.empty((NCORES, PR, DM), np.float16)
        x16 = x.astype(np.float16)
        mz16 = mz[:, :, 0].astype(np.float16)          # (B, L)
        for c in range(NCORES):
            b, s = c // 4, c % 4
            xin[c, 0:L] = x16[b]
            xin[c, L:L + LQ] = x16[b, s * LQ:(s + 1) * LQ]
            xin[c, L + LQ:L + LQ + 2] = mz16[b].reshape(2, DM)
            xin[c, L + LQ + 2, 0:LQ] = mz16[b, s * LQ:(s + 1) * LQ]
        return xin.reshape(NCORES * PR, DM)

    y = runner(make_xin, act_key)

    y = y.reshape(NCORES, LQ, DM)
    out = np.empty((B, L, DM), np.float32)
    for c in range(NCORES):
        b, s = c // 4, c % 4
        out[b, s * LQ:(s + 1) * LQ] = y[c]
    return out
